# revision 8
# baseline (speedup 1.0000x reference)
"""MinGRU cell kernel for Trainium2 (8 NeuronCores, data-parallel over batch).

Computes, for x:[B,T,D], motion_mag:[B,T]:
    tau = 1 + softplus(alpha) * sigmoid(mw*mm + mb)        (per b,t)
    z   = sigmoid((x @ Wz^T + bz) / tau)                   (B,T,H)
    ht  = x @ Wh^T + bh                                    (B,T,H)
    h_t = (1-z_t)*h_{t-1} + z_t*ht_t   (scan over t, h_0=0)

Strategy:
  - Shard B=32 across 8 cores (4 per core). Weights replicated.
  - On-chip layout: h on partitions, t on the free dim, so the recurrence is
    a HW tensor_tensor_scan per [128h, 1024t] tile, carried across t-tiles via
    initial=prev[:, -1:].
  - Projections: lhsT = W^T chunks (stationary), rhs = x^T chunks (moving),
    float32r (full PE rate, near-fp32 accuracy, fp32 PSUM accumulation).
  - Post-GEMM pipeline balanced across engines (DVE is the bottleneck; its
    scan is fixed at 2 cyc/elem, so every other DVE op must run in a bf16
    fast mode and PSUM evacuation goes to ACT):
      ACT     : v = zq + bz   (PSUM->SBUF, bf16)
      DVE TT  : u = v * invtau                  (bf16 SBUF, 2x mode)
      ACT     : z = sigmoid(u)                  (bf16)
      DVE TS  : a = 1 - z                       (bf16, 4x mode)
      ACT     : ht = hq + bh  (PSUM->SBUF)      (bf16 out)
      DVE TT  : b = z * ht                      (bf16, 2x mode)
      DVE scan: h = scan(a, b)                  (bf16 io, fp32 state)
    The output DMA is bf16; the host casts back to fp32.
  - PSUM tiles are 512 wide with 4 buffers per GEMM so the PE can run a full
    tile ahead of the post-GEMM pipeline; emission is software-pipelined
    (tile i's GEMM+ACT front is emitted before tile i-1's DVE back end) so
    the DVE never sits behind a same-tile ACT dependency in its FIFO.
  - A few dummy fp32 matmuls at t=0 warm the PE HAM clock-gate (2.4 GHz)
    while the first weight/x DMAs land.
  - tau: 1/tau computed on host (bf16), DMA-broadcast across partitions.
  - Host pre-transposes x to [d, b*t] per core and un-transposes the output.
"""

import sys

import numpy as np

if "/opt/trn_rl_repo" not in sys.path:
    sys.path.insert(0, "/opt/trn_rl_repo")

B, T, D, H = 32, 2048, 512, 512
NCORES = 8
BL = B // NCORES            # batch per core = 4
TBLK = 1024                 # t-columns per block
MMN = 512                   # matmul free-dim (1 psum bank)
NTB = T // TBLK             # 2 t-blocks per sample
DC = D // 128               # 4 contraction chunks
HC = H // 128               # 4 h partition chunks
BT = BL * T                 # 8192 columns per core

_CACHE = {}


def _build_nc(bz0=None, bh0=None):
    import concourse.bass as bass
    import concourse.bacc as bacc
    import concourse.mybir as mybir
    import concourse.tile as tile
    from contextlib import ExitStack

    f32 = mybir.dt.float32
    f32r = mybir.dt.float32r
    bf16 = mybir.dt.bfloat16
    AF = mybir.ActivationFunctionType
    OP = mybir.AluOpType

    nc = bacc.Bacc("TRN2", target_bir_lowering=False, debug=False)

    xt_ext = nc.declare_dram_parameter("xt", [DC, 128, BT], f32r, isOutput=False)
    wzt_ext = nc.declare_dram_parameter("wzt", [HC, 128, DC, 128], f32r, isOutput=False)
    wht_ext = nc.declare_dram_parameter("wht", [HC, 128, DC, 128], f32r, isOutput=False)
    bz_ext = nc.declare_dram_parameter("bz", [HC, 128, 1], f32, isOutput=False)
    bh_ext = nc.declare_dram_parameter("bh", [HC, 128, 1], f32, isOutput=False)
    itau_ext = nc.declare_dram_parameter("invtau", [BL, 1, T], bf16, isOutput=False)
    out_ext = nc.declare_dram_parameter("out", [BL, HC, 128, T], bf16, isOutput=True)

    with tile.TileContext(nc) as tc, ExitStack() as ctx:
        singles = ctx.enter_context(tc.tile_pool(name="singles", bufs=1))
        x_pool = ctx.enter_context(tc.tile_pool(name="x", bufs=3))
        j_pool = ctx.enter_context(tc.tile_pool(name="j", bufs=3))
        psum = ctx.enter_context(tc.tile_pool(name="psum", bufs=2, space="PSUM"))
        work = ctx.enter_context(tc.tile_pool(name="work", bufs=4))
        ab_pool = ctx.enter_context(tc.tile_pool(name="ab", bufs=4))
        h_pool = ctx.enter_context(tc.tile_pool(name="h", bufs=8))

        # HAM warm-up: a few dependency-free fp32 matmuls (1 col / 4 cycles,
        # so each is long) keep the PE busy while the first weight/x DMAs
        # land, flipping the clock-gate to 8/8 before the real GEMMs start.
        warm = singles.tile([128, MMN], f32, tag="warm", name="warm")
        nc.vector.memset(warm[:], 0.0)
        # Dummy activation: triggers the ~2.7us ACT table load during the
        # initial DMA window instead of on the first tile's critical path.
        warmact = singles.tile([128, 1], bf16, tag="warmact", name="warmact")
        nc.scalar.activation(warmact[:], warm[:, 0:1], AF.Sigmoid)
        wq0 = psum.tile([128, MMN], f32, tag="zq", name="warmq")
        for i in range(3):
            nc.tensor.matmul(
                wq0[:], lhsT=warm[:, 0:128], rhs=warm[:], start=True, stop=True
            )

        # Weights are hc-major in DRAM: the first matmul group (hc=0) only
        # needs a 256KB DMA. First block's x arrives as 512-col halves so the
        # first 4-matmul group is gated on ~1.3MB instead of 3MB.
        wz_hc, wh_hc = [None] * HC, [None] * HC
        xs0h = [[None] * DC for _ in range(2)]
        wz_hc[0] = singles.tile([128, DC * 128], f32r, tag="wzhc0", name="wzhc0")
        nc.sync.dma_start(out=wz_hc[0][:], in_=wzt_ext[0])
        for dc in range(DC):
            xt = x_pool.tile([128, MMN], f32r, tag=f"x{dc}", name=f"x0a_{dc}")
            nc.sync.dma_start(out=xt[:], in_=xt_ext[dc, :, 0:MMN])
            xs0h[0][dc] = xt
        wh_hc[0] = singles.tile([128, DC * 128], f32r, tag="whhc0", name="whhc0")
        nc.sync.dma_start(out=wh_hc[0][:], in_=wht_ext[0])
        for dc in range(DC):
            xt = x_pool.tile([128, MMN], f32r, tag=f"x{dc}", name=f"x0b_{dc}")
            nc.sync.dma_start(out=xt[:], in_=xt_ext[dc, :, MMN:TBLK])
            xs0h[1][dc] = xt
        for hc in range(1, HC):
            w = singles.tile([128, DC * 128], f32r, tag=f"wzhc{hc}", name=f"wzhc{hc}")
            nc.sync.dma_start(out=w[:], in_=wzt_ext[hc])
            wz_hc[hc] = w
            w = singles.tile([128, DC * 128], f32r, tag=f"whhc{hc}", name=f"whhc{hc}")
            nc.sync.dma_start(out=w[:], in_=wht_ext[hc])
            wh_hc[hc] = w
        # gpsimd queue: the first block's 1/tau halves go first; bias columns
        # are DMA'd only when non-uniform (uniform biases ride as immediates).
        jt0 = j_pool.tile([128, TBLK], bf16, tag="J", name="jt0")
        for half in range(2):
            iv0 = itau_ext[0, 0, half * MMN:(half + 1) * MMN]
            iv0_b = bass.AP(
                tensor=iv0.tensor, offset=iv0.offset, ap=[[0, 128]] + list(iv0.ap)
            )
            nc.gpsimd.dma_start(out=jt0[:, half * MMN:(half + 1) * MMN], in_=iv0_b)
        def bias_cols(b0, ext, label):
            # Returns per-hc bias operands for ACT evacuation: 0.0 stays a
            # float (pure Copy); any other value becomes a [128,1] SBUF AP.
            if b0 is not None and b0 == 0.0:
                return [0.0] * HC
            if b0 is not None:
                bc = singles.tile([128, 1], f32, tag=f"{label}u", name=f"{label}u")
                nc.vector.memset(bc[:], b0)
                return [bc[:]] * HC
            cols = []
            for hc in range(HC):
                bc = singles.tile([128, 1], f32, tag=f"{label}{hc}", name=f"{label}{hc}")
                nc.gpsimd.dma_start(out=bc[:], in_=ext[hc])
                cols.append(bc[:])
            return cols

        bz_col = bias_cols(bz0, bz_ext, "bz")
        bh_col = bias_cols(bh0, bh_ext, "bh")

        h_prev = [[None] * HC for _ in range(BL)]

        def evac(dst, src_q, bias):
            # PSUM -> SBUF bf16 evacuation with per-partition bias on ACT.
            if isinstance(bias, float):
                if bias == 0.0:
                    nc.scalar.activation(dst, src_q, AF.Copy)
                else:
                    nc.scalar.activation(dst, src_q, AF.Identity, bias=bias)
            else:
                nc.scalar.activation(dst, src_q, AF.Identity, bias=bias)

        def emit_front(b, tb, hc, xs, jt, nsub):
            """GEMMs + ACT evacuations + u/sigmoid/a for one [128,1024] tile."""
            first_blk = (b == 0 and tb == 0)
            v = work.tile([128, TBLK], bf16, tag="v")
            u = work.tile([128, TBLK], bf16, tag="u")
            z = work.tile([128, TBLK], bf16, tag="z")
            a = ab_pool.tile([128, TBLK], bf16, tag="a")
            ht = ab_pool.tile([128, TBLK], bf16, tag="ht")
            width = TBLK // nsub
            zq = psum.tile([128, TBLK], f32, tag="zq")
            for half in range(2):
                psl = slice(half * MMN, (half + 1) * MMN)
                for dc in range(DC):
                    rhs = (xs0h[half][dc][:] if first_blk else xs[dc][:, psl])
                    nc.tensor.matmul(
                        zq[:, psl],
                        lhsT=wz_hc[hc][:, dc * 128:(dc + 1) * 128],
                        rhs=rhs,
                        start=(dc == 0),
                        stop=(dc == DC - 1),
                    )
            hq = psum.tile([128, TBLK], f32, tag="hq")
            for half in range(2):
                psl = slice(half * MMN, (half + 1) * MMN)
                for dc in range(DC):
                    rhs = (xs0h[half][dc][:] if first_blk else xs[dc][:, psl])
                    nc.tensor.matmul(
                        hq[:, psl],
                        lhsT=wh_hc[hc][:, dc * 128:(dc + 1) * 128],
                        rhs=rhs,
                        start=(dc == 0),
                        stop=(dc == DC - 1),
                    )
            for sub in range(nsub):
                ssl = slice(sub * width, (sub + 1) * width)
                # v = zpre + bz  (ACT, frees the z psum)
                evac(v[:, ssl], zq[:, ssl], bz_col[hc])
                # u = v * invtau  (DVE bf16 2x)
                nc.vector.tensor_tensor(u[:, ssl], v[:, ssl], jt[:, ssl], OP.mult)
                # z = sigmoid(u)  (ACT)
                nc.scalar.activation(z[:, ssl], u[:, ssl], AF.Sigmoid)
                # a = 1 - z  (GPSIMD; keeps the DVE free for the scan)
                nc.gpsimd.tensor_scalar(
                    a[:, ssl], z[:, ssl], -1.0, 1.0, op0=OP.mult, op1=OP.add
                )
                # ht = hpre + bh  (ACT, frees the h psum)
                evac(ht[:, ssl], hq[:, ssl], bh_col[hc])
            return (b, tb, hc, z, a, ht, nsub)

        def emit_back(desc):
            """b/scan/out-DMA for a tile whose front was already emitted."""
            b, tb, hc, z, a, ht, nsub = desc
            bb = ab_pool.tile([128, TBLK], bf16, tag="b")
            h = h_pool.tile([128, TBLK], bf16, tag="h")
            width = TBLK // nsub
            for sub in range(nsub):
                ssl = slice(sub * width, (sub + 1) * width)
                # b = z * ht  (DVE bf16 2x)
                nc.vector.tensor_tensor(bb[:, ssl], z[:, ssl], ht[:, ssl], OP.mult)
                init = (
                    (0.0 if tb == 0 else h_prev[b][hc][:, TBLK - 1:TBLK])
                    if sub == 0 else h[:, sub * width - 1:sub * width]
                )
                nc.vector.tensor_tensor_scan(
                    h[:, ssl], a[:, ssl], bb[:, ssl], init,
                    op0=OP.mult, op1=OP.add,
                )
                if nsub > 1:
                    osl = slice(tb * TBLK + sub * width,
                                tb * TBLK + (sub + 1) * width)
                    nc.sync.dma_start(out=out_ext[b, hc, :, osl], in_=h[:, ssl])
            h_prev[b][hc] = h
            if nsub == 1:
                ts = slice(tb * TBLK, (tb + 1) * TBLK)
                nc.sync.dma_start(out=out_ext[b, hc, :, ts], in_=h[:])

        # 1/tau broadcast tiles, prefetched one block ahead so the gpsimd
        # queue (which also serves the `a` ops) never gates a block start.
        jt_tiles = [None] * (BL * NTB)
        jt_tiles[0] = jt0

        def emit_jt(k):
            b_, tb_ = divmod(k, NTB)
            jt = j_pool.tile([128, TBLK], bf16, tag="J", name=f"jt{k}")
            iv = itau_ext[b_, 0, tb_ * TBLK:(tb_ + 1) * TBLK]
            iv_b = bass.AP(
                tensor=iv.tensor, offset=iv.offset,
                ap=[[0, 128]] + list(iv.ap),
            )
            nc.gpsimd.dma_start(out=jt[:], in_=iv_b)
            jt_tiles[k] = jt

        pending = None
        for b in range(BL):
            for tb in range(NTB):
                k = b * NTB + tb
                bt0 = b * T + tb * TBLK
                first_blk = (k == 0)
                if k + 1 < BL * NTB:
                    emit_jt(k + 1)
                if first_blk:
                    xs = None
                else:
                    xs = []
                    for dc in range(DC):
                        xt = x_pool.tile([128, TBLK], f32r, tag=f"x{dc}")
                        nc.sync.dma_start(
                            out=xt[:], in_=xt_ext[dc, :, bt0:bt0 + TBLK]
                        )
                        xs.append(xt)
                jt = jt_tiles[k]

                for hc in range(HC):
                    last_blk = (b == BL - 1 and tb == NTB - 1 and hc == HC - 1)
                    nsub = 2 if ((first_blk and hc == 0) or last_blk) else 1
                    front = emit_front(b, tb, hc, xs, jt, nsub)
                    if pending is not None:
                        emit_back(pending)
                    pending = front
        emit_back(pending)

    nc.compile()
    return nc


def _prep_inputs(x, motion_mag, Wz, bz, Wh, bh, motion_weight, motion_bias, alpha):
    import ml_dtypes

    x = np.ascontiguousarray(np.asarray(x, dtype=np.float32))
    mm = np.asarray(motion_mag, dtype=np.float32)
    Wz = np.asarray(Wz, dtype=np.float32)
    Wh = np.asarray(Wh, dtype=np.float32)
    bz = np.asarray(bz, dtype=np.float32).reshape(HC, 128, 1)
    bh = np.asarray(bh, dtype=np.float32).reshape(HC, 128, 1)
    mw = float(np.asarray(motion_weight))
    mb = float(np.asarray(motion_bias))
    al = float(np.asarray(alpha))

    a_sp = float(np.log1p(np.exp(al)))  # softplus(alpha)
    sig = 1.0 / (1.0 + np.exp(-(mw * mm + mb)))
    invtau = (1.0 / (1.0 + a_sp * sig)).astype(ml_dtypes.bfloat16)

    wzt = np.ascontiguousarray(
        Wz.T.reshape(DC, 128, HC, 128).transpose(2, 1, 0, 3))
    wht = np.ascontiguousarray(
        Wh.T.reshape(DC, 128, HC, 128).transpose(2, 1, 0, 3))

    in_maps = []
    for c in range(NCORES):
        xl = x[c * BL:(c + 1) * BL].reshape(BL * T, D)
        xt = np.ascontiguousarray(xl.T).reshape(DC, 128, BT)
        in_maps.append({
            "xt": xt,
            "wzt": wzt,
            "wht": wht,
            "bz": bz,
            "bh": bh,
            "invtau": np.ascontiguousarray(
                invtau[c * BL:(c + 1) * BL]).reshape(BL, 1, T),
        })
    return in_maps


def _assemble(results):
    outs = []
    for c in range(NCORES):
        o = results[c]["out"]  # [BL, HC, 128, T] bf16
        o = np.transpose(o.astype(np.float32), (0, 3, 1, 2)).reshape(BL, T, H)
        outs.append(o)
    return np.ascontiguousarray(np.concatenate(outs, axis=0))


def _run(inputs, trace=False):
    from concourse.bass_utils import run_bass_kernel_spmd

    bza = np.asarray(inputs["bz"], dtype=np.float32).reshape(-1)
    bha = np.asarray(inputs["bh"], dtype=np.float32).reshape(-1)
    bz0 = float(bza[0]) if np.all(bza == bza[0]) else None
    bh0 = float(bha[0]) if np.all(bha == bha[0]) else None
    key = ("nc", bz0, bh0)
    if key not in _CACHE:
        _CACHE[key] = _build_nc(bz0, bh0)
    nc = _CACHE[key]
    in_maps = _prep_inputs(**inputs)
    res = run_bass_kernel_spmd(nc, in_maps, list(range(NCORES)), trace=trace)
    return _assemble(res.results), res


def kernel(**inputs):
    out, _ = _run(inputs, trace=False)
    return out


# revision 9
# speedup vs baseline: 1.1134x; 1.1134x over previous
"""MinGRU cell kernel for Trainium2 (8 NeuronCores, data-parallel over batch).

Computes, for x:[B,T,D], motion_mag:[B,T]:
    tau = 1 + softplus(alpha) * sigmoid(mw*mm + mb)        (per b,t)
    z   = sigmoid((x @ Wz^T + bz) / tau)                   (B,T,H)
    ht  = x @ Wh^T + bh                                    (B,T,H)
    h_t = (1-z_t)*h_{t-1} + z_t*ht_t   (scan over t, h_0=0)

Strategy:
  - Shard B=32 across 8 cores (4 per core). Weights replicated.
  - On-chip layout: h on partitions, t on the free dim, so the recurrence is
    a HW tensor_tensor_scan per [128h, 1024t] tile, carried across t-tiles via
    initial=prev[:, -1:].
  - Projections: lhsT = W^T chunks (stationary), rhs = x^T chunks (moving),
    float32r (full PE rate, near-fp32 accuracy, fp32 PSUM accumulation).
  - Post-GEMM pipeline balanced across engines (DVE is the bottleneck; its
    scan is fixed at 2 cyc/elem, so every other DVE op must run in a bf16
    fast mode and PSUM evacuation goes to ACT):
      ACT     : v = zq + bz   (PSUM->SBUF, bf16)
      DVE TT  : u = v * invtau                  (bf16 SBUF, 2x mode)
      ACT     : z = sigmoid(u)                  (bf16)
      DVE TS  : a = 1 - z                       (bf16, 4x mode)
      ACT     : ht = hq + bh  (PSUM->SBUF)      (bf16 out)
      DVE TT  : b = z * ht                      (bf16, 2x mode)
      DVE scan: h = scan(a, b)                  (bf16 io, fp32 state)
    The output DMA is bf16; the host casts back to fp32.
  - PSUM tiles are 512 wide with 4 buffers per GEMM so the PE can run a full
    tile ahead of the post-GEMM pipeline; emission is software-pipelined
    (tile i's GEMM+ACT front is emitted before tile i-1's DVE back end) so
    the DVE never sits behind a same-tile ACT dependency in its FIFO.
  - A few dummy fp32 matmuls at t=0 warm the PE HAM clock-gate (2.4 GHz)
    while the first weight/x DMAs land.
  - tau: 1/tau computed on host (bf16), DMA-broadcast across partitions.
  - Host pre-transposes x to [d, b*t] per core and un-transposes the output.
"""

import sys

import numpy as np

if "/opt/trn_rl_repo" not in sys.path:
    sys.path.insert(0, "/opt/trn_rl_repo")

B, T, D, H = 32, 2048, 512, 512
NCORES = 8
BL = B // NCORES            # batch per core = 4
TBLK = 1024                 # t-columns per block
MMN = 512                   # matmul free-dim (1 psum bank)
NTB = T // TBLK             # 2 t-blocks per sample
DC = D // 128               # 4 contraction chunks
HC = H // 128               # 4 h partition chunks
BT = BL * T                 # 8192 columns per core

_CACHE = {}


def _build_nc(bz0=None, bh0=None):
    import concourse.bass as bass
    import concourse.bacc as bacc
    import concourse.mybir as mybir
    import concourse.tile as tile
    from contextlib import ExitStack

    f32 = mybir.dt.float32
    f32r = mybir.dt.float32r
    bf16 = mybir.dt.bfloat16
    AF = mybir.ActivationFunctionType
    OP = mybir.AluOpType

    nc = bacc.Bacc("TRN2", target_bir_lowering=False, debug=False)

    xt_ext = nc.declare_dram_parameter("xt", [DC, 128, BT], f32r, isOutput=False)
    wzt_ext = nc.declare_dram_parameter("wzt", [HC, 128, DC, 128], f32r, isOutput=False)
    wht_ext = nc.declare_dram_parameter("wht", [HC, 128, DC, 128], f32r, isOutput=False)
    bz_ext = nc.declare_dram_parameter("bz", [HC, 128, 1], f32, isOutput=False)
    bh_ext = nc.declare_dram_parameter("bh", [HC, 128, 1], f32, isOutput=False)
    itau_ext = nc.declare_dram_parameter("invtau", [BL, 1, T], bf16, isOutput=False)
    out_ext = nc.declare_dram_parameter("out", [BL, HC, 128, T], bf16, isOutput=True)

    with tile.TileContext(nc) as tc, ExitStack() as ctx:
        singles = ctx.enter_context(tc.tile_pool(name="singles", bufs=1))
        x_pool = ctx.enter_context(tc.tile_pool(name="x", bufs=3))
        j_pool = ctx.enter_context(tc.tile_pool(name="j", bufs=3))
        psum = ctx.enter_context(tc.tile_pool(name="psum", bufs=2, space="PSUM"))
        work = ctx.enter_context(tc.tile_pool(name="work", bufs=4))
        ab_pool = ctx.enter_context(tc.tile_pool(name="ab", bufs=4))
        h_pool = ctx.enter_context(tc.tile_pool(name="h", bufs=8))

        # HAM warm-up: a few dependency-free fp32 matmuls (1 col / 4 cycles,
        # so each is long) keep the PE busy while the first weight/x DMAs
        # land, flipping the clock-gate to 8/8 before the real GEMMs start.
        warm = singles.tile([128, MMN], f32, tag="warm", name="warm")
        nc.vector.memset(warm[:], 0.0)
        # Dummy activation: triggers the ~2.7us ACT table load during the
        # initial DMA window instead of on the first tile's critical path.
        warmact = singles.tile([128, 1], bf16, tag="warmact", name="warmact")
        nc.scalar.activation(warmact[:], warm[:, 0:1], AF.Sigmoid)
        wq0 = psum.tile([128, MMN], f32, tag="zq", name="warmq")
        for i in range(3):
            nc.tensor.matmul(
                wq0[:], lhsT=warm[:, 0:128], rhs=warm[:], start=True, stop=True
            )

        # Weights are hc-major in DRAM: the first matmul group (hc=0) only
        # needs a 256KB DMA. First block's x arrives as 512-col halves so the
        # first 4-matmul group is gated on ~1.3MB instead of 3MB.
        wz_hc, wh_hc = [None] * HC, [None] * HC
        xs0h = [[None] * DC for _ in range(2)]
        wz_hc[0] = singles.tile([128, DC * 128], f32r, tag="wzhc0", name="wzhc0")
        nc.sync.dma_start(out=wz_hc[0][:], in_=wzt_ext[0])
        for dc in range(DC):
            xt = x_pool.tile([128, MMN], f32r, tag=f"x{dc}", name=f"x0a_{dc}")
            nc.sync.dma_start(out=xt[:], in_=xt_ext[dc, :, 0:MMN])
            xs0h[0][dc] = xt
        wh_hc[0] = singles.tile([128, DC * 128], f32r, tag="whhc0", name="whhc0")
        nc.sync.dma_start(out=wh_hc[0][:], in_=wht_ext[0])
        for dc in range(DC):
            xt = x_pool.tile([128, MMN], f32r, tag=f"x{dc}", name=f"x0b_{dc}")
            nc.sync.dma_start(out=xt[:], in_=xt_ext[dc, :, MMN:TBLK])
            xs0h[1][dc] = xt
        for hc in range(1, HC):
            w = singles.tile([128, DC * 128], f32r, tag=f"wzhc{hc}", name=f"wzhc{hc}")
            nc.sync.dma_start(out=w[:], in_=wzt_ext[hc])
            wz_hc[hc] = w
            w = singles.tile([128, DC * 128], f32r, tag=f"whhc{hc}", name=f"whhc{hc}")
            nc.sync.dma_start(out=w[:], in_=wht_ext[hc])
            wh_hc[hc] = w
        # gpsimd queue: the first block's 1/tau halves go first; bias columns
        # are DMA'd only when non-uniform (uniform biases ride as immediates).
        jt0 = j_pool.tile([128, TBLK], bf16, tag="J", name="jt0")
        for half in range(2):
            iv0 = itau_ext[0, 0, half * MMN:(half + 1) * MMN]
            iv0_b = bass.AP(
                tensor=iv0.tensor, offset=iv0.offset, ap=[[0, 128]] + list(iv0.ap)
            )
            nc.gpsimd.dma_start(out=jt0[:, half * MMN:(half + 1) * MMN], in_=iv0_b)
        def bias_cols(b0, ext, label):
            # Returns per-hc bias operands for ACT evacuation: 0.0 stays a
            # float (pure Copy); any other value becomes a [128,1] SBUF AP.
            if b0 is not None and b0 == 0.0:
                return [0.0] * HC
            if b0 is not None:
                bc = singles.tile([128, 1], f32, tag=f"{label}u", name=f"{label}u")
                nc.vector.memset(bc[:], b0)
                return [bc[:]] * HC
            cols = []
            for hc in range(HC):
                bc = singles.tile([128, 1], f32, tag=f"{label}{hc}", name=f"{label}{hc}")
                nc.gpsimd.dma_start(out=bc[:], in_=ext[hc])
                cols.append(bc[:])
            return cols

        bz_col = bias_cols(bz0, bz_ext, "bz")
        bh_col = bias_cols(bh0, bh_ext, "bh")

        h_prev = [[None] * HC for _ in range(BL)]

        def evac(dst, src_q, bias):
            # PSUM -> SBUF bf16 evacuation with per-partition bias on ACT.
            if isinstance(bias, float):
                if bias == 0.0:
                    nc.scalar.activation(dst, src_q, AF.Copy)
                else:
                    nc.scalar.activation(dst, src_q, AF.Identity, bias=bias)
            else:
                nc.scalar.activation(dst, src_q, AF.Identity, bias=bias)

        # Tile schedule: 32 tiles of [128h, 1024t], hc-inner.
        sched = []
        for b in range(BL):
            for tb in range(NTB):
                for hc in range(HC):
                    first_blk = (b == 0 and tb == 0)
                    last = (b == BL - 1 and tb == NTB - 1 and hc == HC - 1)
                    nsub = 2 if ((first_blk and hc == 0) or last) else 1
                    sched.append((b, tb, hc, nsub))
        NT = len(sched)

        # 1/tau broadcast tiles, prefetched one block ahead so the gpsimd
        # DMA queue never gates a block start.
        jt_tiles = [None] * (BL * NTB)
        jt_tiles[0] = jt0

        def emit_jt(k):
            b_, tb_ = divmod(k, NTB)
            jt = j_pool.tile([128, TBLK], bf16, tag="J", name=f"jt{k}")
            iv = itau_ext[b_, 0, tb_ * TBLK:(tb_ + 1) * TBLK]
            iv_b = bass.AP(
                tensor=iv.tensor, offset=iv.offset,
                ap=[[0, 128]] + list(iv.ap),
            )
            nc.gpsimd.dma_start(out=jt[:], in_=iv_b)
            jt_tiles[k] = jt

        xs_blocks = {}
        state = [None] * NT
        h_prev = [[None] * HC for _ in range(BL)]

        def emit_mms(i):
            """x DMAs (at block starts) + both GEMMs for tile i."""
            b, tb, hc, nsub = sched[i]
            k = b * NTB + tb
            first_blk = (k == 0)
            if hc == 0 and not first_blk:
                bt0 = b * T + tb * TBLK
                xs = []
                for dc in range(DC):
                    xt = x_pool.tile([128, TBLK], f32r, tag=f"x{dc}")
                    nc.sync.dma_start(out=xt[:], in_=xt_ext[dc, :, bt0:bt0 + TBLK])
                    xs.append(xt)
                xs_blocks[k] = xs
            if hc == 0 and k + 1 < BL * NTB:
                emit_jt(k + 1)
            xs = None if first_blk else xs_blocks[k]
            zq = psum.tile([128, TBLK], f32, tag="zq")
            for half in range(2):
                psl = slice(half * MMN, (half + 1) * MMN)
                for dc in range(DC):
                    rhs = (xs0h[half][dc][:] if first_blk else xs[dc][:, psl])
                    nc.tensor.matmul(
                        zq[:, psl],
                        lhsT=wz_hc[hc][:, dc * 128:(dc + 1) * 128],
                        rhs=rhs,
                        start=(dc == 0),
                        stop=(dc == DC - 1),
                    )
            hq = psum.tile([128, TBLK], f32, tag="hq")
            for half in range(2):
                psl = slice(half * MMN, (half + 1) * MMN)
                for dc in range(DC):
                    rhs = (xs0h[half][dc][:] if first_blk else xs[dc][:, psl])
                    nc.tensor.matmul(
                        hq[:, psl],
                        lhsT=wh_hc[hc][:, dc * 128:(dc + 1) * 128],
                        rhs=rhs,
                        start=(dc == 0),
                        stop=(dc == DC - 1),
                    )
            state[i] = {"zq": zq, "hq": hq}

        def emit_evacs(i):
            """ACT: v = zq + bz, ht = hq + bh (PSUM -> SBUF bf16)."""
            b, tb, hc, nsub = sched[i]
            st = state[i]
            v = work.tile([128, TBLK], bf16, tag="v")
            ht = ab_pool.tile([128, TBLK], bf16, tag="ht")
            width = TBLK // nsub
            for sub in range(nsub):
                ssl = slice(sub * width, (sub + 1) * width)
                evac(v[:, ssl], st["zq"][:, ssl], bz_col[hc])
                evac(ht[:, ssl], st["hq"][:, ssl], bh_col[hc])
            st["v"], st["ht"] = v, ht

        def emit_usigma(i):
            """DVE: u = v * invtau (bf16 2x); ACT: z = sigmoid(u)."""
            b, tb, hc, nsub = sched[i]
            st = state[i]
            jt = jt_tiles[b * NTB + tb]
            u = work.tile([128, TBLK], bf16, tag="u")
            z = work.tile([128, TBLK], bf16, tag="z")
            width = TBLK // nsub
            for sub in range(nsub):
                ssl = slice(sub * width, (sub + 1) * width)
                nc.vector.tensor_tensor(
                    u[:, ssl], st["v"][:, ssl], jt[:, ssl], OP.mult
                )
                nc.scalar.activation(z[:, ssl], u[:, ssl], AF.Sigmoid)
            st["z"] = z

        def emit_back(i):
            """DVE: a = 1-z (4x), b = z*ht (2x), scan, out-DMA."""
            b, tb, hc, nsub = sched[i]
            st = state[i]
            z, ht = st["z"], st["ht"]
            a = ab_pool.tile([128, TBLK], bf16, tag="a")
            bb = ab_pool.tile([128, TBLK], bf16, tag="b")
            h = h_pool.tile([128, TBLK], bf16, tag="h")
            width = TBLK // nsub
            for sub in range(nsub):
                ssl = slice(sub * width, (sub + 1) * width)
                nc.vector.tensor_scalar(
                    a[:, ssl], z[:, ssl], -1.0, 1.0, op0=OP.mult, op1=OP.add
                )
                nc.vector.tensor_tensor(
                    bb[:, ssl], z[:, ssl], ht[:, ssl], OP.mult
                )
                init = (
                    (0.0 if tb == 0 else h_prev[b][hc][:, TBLK - 1:TBLK])
                    if sub == 0 else h[:, sub * width - 1:sub * width]
                )
                nc.vector.tensor_tensor_scan(
                    h[:, ssl], a[:, ssl], bb[:, ssl], init,
                    op0=OP.mult, op1=OP.add,
                )
                if nsub > 1:
                    osl = slice(tb * TBLK + sub * width,
                                tb * TBLK + (sub + 1) * width)
                    nc.sync.dma_start(out=out_ext[b, hc, :, osl], in_=h[:, ssl])
            h_prev[b][hc] = h
            if nsub == 1:
                ts = slice(tb * TBLK, (tb + 1) * TBLK)
                nc.sync.dma_start(out=out_ext[b, hc, :, ts], in_=h[:])
            state[i] = None

        # Software-pipelined emission. Per-engine FIFO orders in steady state:
        #   Tensor: MM(i+1) stream (one tile ahead of the consumers)
        #   DVE   : u(i), a(i-1), b(i-1), scan(i-1)
        #   ACT   : sigma(i), v(i+1), ht(i+1)
        # This keeps the loop-carried cross-engine cycle
        # u(i)->sigma(i)->v(i+1)->u(i+1) well below the DVE period, so the
        # DVE (scan-dominated) paces the kernel with positive slack on every
        # cross-engine edge.
        emit_mms(0)
        emit_evacs(0)
        for i in range(NT):
            if i + 1 < NT:
                emit_mms(i + 1)
            emit_usigma(i)
            if i - 1 >= 0:
                emit_back(i - 1)
            if i + 1 < NT:
                emit_evacs(i + 1)
        emit_back(NT - 1)

    nc.compile()
    return nc


def _prep_inputs(x, motion_mag, Wz, bz, Wh, bh, motion_weight, motion_bias, alpha):
    import ml_dtypes

    x = np.ascontiguousarray(np.asarray(x, dtype=np.float32))
    mm = np.asarray(motion_mag, dtype=np.float32)
    Wz = np.asarray(Wz, dtype=np.float32)
    Wh = np.asarray(Wh, dtype=np.float32)
    bz = np.asarray(bz, dtype=np.float32).reshape(HC, 128, 1)
    bh = np.asarray(bh, dtype=np.float32).reshape(HC, 128, 1)
    mw = float(np.asarray(motion_weight))
    mb = float(np.asarray(motion_bias))
    al = float(np.asarray(alpha))

    a_sp = float(np.log1p(np.exp(al)))  # softplus(alpha)
    sig = 1.0 / (1.0 + np.exp(-(mw * mm + mb)))
    invtau = (1.0 / (1.0 + a_sp * sig)).astype(ml_dtypes.bfloat16)

    wzt = np.ascontiguousarray(
        Wz.T.reshape(DC, 128, HC, 128).transpose(2, 1, 0, 3))
    wht = np.ascontiguousarray(
        Wh.T.reshape(DC, 128, HC, 128).transpose(2, 1, 0, 3))

    in_maps = []
    for c in range(NCORES):
        xl = x[c * BL:(c + 1) * BL].reshape(BL * T, D)
        xt = np.ascontiguousarray(xl.T).reshape(DC, 128, BT)
        in_maps.append({
            "xt": xt,
            "wzt": wzt,
            "wht": wht,
            "bz": bz,
            "bh": bh,
            "invtau": np.ascontiguousarray(
                invtau[c * BL:(c + 1) * BL]).reshape(BL, 1, T),
        })
    return in_maps


def _assemble(results):
    outs = []
    for c in range(NCORES):
        o = results[c]["out"]  # [BL, HC, 128, T] bf16
        o = np.transpose(o.astype(np.float32), (0, 3, 1, 2)).reshape(BL, T, H)
        outs.append(o)
    return np.ascontiguousarray(np.concatenate(outs, axis=0))


def _run(inputs, trace=False):
    from concourse.bass_utils import run_bass_kernel_spmd

    bza = np.asarray(inputs["bz"], dtype=np.float32).reshape(-1)
    bha = np.asarray(inputs["bh"], dtype=np.float32).reshape(-1)
    bz0 = float(bza[0]) if np.all(bza == bza[0]) else None
    bh0 = float(bha[0]) if np.all(bha == bha[0]) else None
    key = ("nc", bz0, bh0)
    if key not in _CACHE:
        _CACHE[key] = _build_nc(bz0, bh0)
    nc = _CACHE[key]
    in_maps = _prep_inputs(**inputs)
    res = run_bass_kernel_spmd(nc, in_maps, list(range(NCORES)), trace=trace)
    return _assemble(res.results), res


def kernel(**inputs):
    out, _ = _run(inputs, trace=False)
    return out
